# revision 7
# baseline (speedup 1.0000x reference)
"""Bahdanau attention forward on 8 Trainium2 NeuronCores.

Reference (per example b):
    q_proj = query[b] @ W1 + b1                      # [U]
    v_proj = values[b] @ W2 + b2                     # [S, U]
    h      = tanh(q_proj + v_proj)                   # [S, U]
    scores = h @ V + bv                              # [S]
    attn   = softmax(scores)                         # [S]
    out    = attn @ values[b]                        # [D]

Shapes: B=64, S=2048, D=512, U=512, fp32.

Sharding: data-parallel over batch. Each of the 8 cores processes 8
examples; W1/W2/V/biases are replicated. No cross-core communication.

Both contractions need different values layouts ([d, s] for v_proj,
[s, d] for the context reduction), so the host passes values twice:
natural and pre-transposed (pure layout prep, no arithmetic). In bf16
mode both copies together cost the same DMA bytes as one fp32 copy,
and the PE runs matmuls at full rate with fast weight load; all PSUM
accumulation and the softmax stay fp32.

Per-core dataflow, software-pipelined one example deep so the PE never
waits on a softmax:
  iter b:   DMA values[b] (natural + transposed)
            v_projT[u,s] matmuls -> tanh(+q_projT bias) -> hT -> scores
            attention transposes + context matmuls for example b-1
            softmax pieces for b (overlap next iteration's PE work)
"""

import os
import sys

sys.path.insert(0, "/opt/trn_rl_repo")

import ml_dtypes
import numpy as np

import concourse.bass as bass
import concourse.tile as tile
from concourse import bacc, mybir
from concourse.bass_utils import run_bass_kernel_spmd

F32 = mybir.dt.float32
AFT = mybir.ActivationFunctionType

NCORES = 8
B, S, D, U = 64, 2048, 512, 512
BC = B // NCORES          # examples per core
T = S // 128              # s-tiles per example
CH = 512                  # s-chunk width (one PSUM bank)
C = S // CH               # s-chunks per example
KD = D // 128             # d-tiles (contraction for v_proj)
KU = U // 128             # u-tiles (contraction for scores)

# Matmul-path dtype. bf16 halves DMA and runs the PE at full rate with
# fast weight load; float32r keeps ~1e-4 accuracy but pays a serialized
# LDWEIGHTS per matmul; float32 is the exact fallback at 1/4 rate.
MODE = os.environ.get("BAH_MODE", "bf16")
WD = {"bf16": mybir.dt.bfloat16,
      "f32r": mybir.dt.float32r,
      "f32": F32}[MODE]
WD_NP = {"bf16": ml_dtypes.bfloat16,
         "f32r": np.float32,
         "f32": np.float32}[MODE]


def build_kernel() -> bass.Bass:
    nc = bacc.Bacc("TRN2", target_bir_lowering=False, debug=False,
                   num_devices=NCORES)

    values_d = nc.dram_tensor("values", [BC, S, D], WD, kind="ExternalInput")
    valuesT_d = nc.dram_tensor("valuesT", [BC, D, S], WD, kind="ExternalInput")
    qT_d = nc.dram_tensor("qT", [D, BC], WD, kind="ExternalInput")
    w1_d = nc.dram_tensor("W1", [D, U], WD, kind="ExternalInput")
    w2_d = nc.dram_tensor("W2", [D, U], WD, kind="ExternalInput")
    v_d = nc.dram_tensor("V", [U], WD, kind="ExternalInput")
    b1_d = nc.dram_tensor("b1", [U], F32, kind="ExternalInput")
    b2_d = nc.dram_tensor("b2", [U], F32, kind="ExternalInput")
    bv_d = nc.dram_tensor("bv", [1, 1], F32, kind="ExternalInput")
    out_d = nc.dram_tensor("out", [BC, D], F32, kind="ExternalOutput")

    big = MODE != "bf16"      # fp32-sized values tiles: tighter SBUF budget

    with tile.TileContext(nc) as tc:
        with tc.tile_pool(name="const", bufs=1) as cpool:
            # Replicated params, laid out for direct use as matmul operands.
            w2_sb = cpool.tile([128, KD, U], WD)
            nc.sync.dma_start(w2_sb[:], w2_d.ap().rearrange("(k p) u -> p k u", p=128))
            w1_sb = cpool.tile([128, KD, U], WD)
            nc.scalar.dma_start(w1_sb[:], w1_d.ap().rearrange("(k p) u -> p k u", p=128))
            qT_sb = cpool.tile([128, KD, BC], WD)
            nc.scalar.dma_start(qT_sb[:], qT_d.ap().rearrange("(k p) b -> p k b", p=128))
            v_sb = cpool.tile([128, KU], WD)
            nc.scalar.dma_start(v_sb[:], v_d.ap().rearrange("(k p) -> p k", p=128))
            b1T = cpool.tile([128, KU], F32)
            nc.scalar.dma_start(b1T[:], b1_d.ap().rearrange("(k p) -> p k", p=128))
            b2T = cpool.tile([128, KU], F32)
            nc.scalar.dma_start(b2T[:], b2_d.ap().rearrange("(k p) -> p k", p=128))
            bv_sb = cpool.tile([1, 1], F32)
            nc.scalar.dma_start(bv_sb[:], bv_d.ap())
            ones = cpool.tile([1, 1], WD)
            nc.vector.memset(ones[:], 1.0)

            b12T = cpool.tile([128, KU], F32)
            nc.vector.tensor_add(b12T[:], b1T[:], b2T[:])

            # q_projT[u, b] + b1 + b2, one [128, BC] tile per u-tile.
            qpbT = cpool.tile([128, KU, BC], F32)
            with tc.tile_pool(name="qp_ps", bufs=2, space="PSUM") as qp_pool:
                for ku in range(KU):
                    qp = qp_pool.tile([128, BC], F32, tag="qp")
                    for kd in range(KD):
                        nc.tensor.matmul(
                            qp[:],
                            w1_sb[:, kd, ku * 128:(ku + 1) * 128],
                            qT_sb[:, kd, :],
                            start=(kd == 0), stop=(kd == KD - 1),
                        )
                    nc.vector.tensor_scalar_add(
                        qpbT[:, ku, :], qp[:], b12T[:, ku:ku + 1])

            with (
                tc.tile_pool(name="vn", bufs=2) as vn_pool,
                tc.tile_pool(name="vT", bufs=1 if big else 2) as vT_pool,
                tc.tile_pool(name="ht", bufs=8) as ht_pool,
                tc.tile_pool(name="rows", bufs=2) as row_pool,
                tc.tile_pool(name="small", bufs=2) as sm_pool,
                tc.tile_pool(name="hp_ps", bufs=2, space="PSUM") as hp_ps,
                tc.tile_pool(name="sc_ps", bufs=2, space="PSUM") as sc_ps,
                tc.tile_pool(name="mi_ps", bufs=2, space="PSUM") as mi_ps,
            ):
                prev = None
                for b in range(BC + 1):
                    if b < BC:
                        # --- load values[b], both layouts ---
                        vT = vT_pool.tile([128, KD, S], WD, tag="vT")
                        vT_src = valuesT_d.ap()[b].rearrange(
                            "(k p) s -> p k s", p=128)
                        for kd in range(KD):
                            nc.sync.dma_start(
                                vT[:, kd, :], vT_src[:, kd, :])
                        vn = vn_pool.tile([128, T, D], WD, tag="vn")
                        nc.sync.dma_start(
                            vn[:],
                            values_d.ap()[b].rearrange("(t p) d -> p t d", p=128))

                        # --- v_projT -> tanh -> hT -> scores ---
                        sc_row = row_pool.tile([1, S], F32, tag="sc")
                        for c2 in range(C // 2):
                            hts = []
                            for ku in range(KU):
                                # two s-chunks share one 2-bank PSUM tile so
                                # the tanh runs once at FD=1024
                                hp = hp_ps.tile([128, 2 * CH], F32, tag="hp")
                                for kd in range(KD):
                                    for h in range(2):
                                        nc.tensor.matmul(
                                            hp[:, h * CH:(h + 1) * CH],
                                            w2_sb[:, kd, ku * 128:(ku + 1) * 128],
                                            vT[:, kd, (2 * c2 + h) * CH:
                                               (2 * c2 + h + 1) * CH],
                                            start=(kd == 0), stop=(kd == KD - 1),
                                        )
                                ht = ht_pool.tile([128, 2 * CH], WD, tag="ht")
                                nc.scalar.activation(
                                    ht[:], hp[:], AFT.Tanh,
                                    bias=qpbT[:, ku, b:b + 1])
                                hts.append(ht)
                            for h in range(2):
                                c = 2 * c2 + h
                                sp = sc_ps.tile([1, CH], F32, tag="sp")
                                for ku in range(KU):
                                    nc.tensor.matmul(
                                        sp[:], v_sb[:, ku:ku + 1],
                                        hts[ku][:, h * CH:(h + 1) * CH],
                                        start=(ku == 0), stop=(ku == KU - 1),
                                    )
                                nc.vector.tensor_scalar_add(
                                    sc_row[:, c * CH:(c + 1) * CH], sp[:],
                                    bv_sb[0:1, 0:1])

                    if prev is not None:
                        # ==== attention + context for example b-1 ====
                        # (softmax for b-1 completed during this iteration's
                        # v_proj matmuls; consuming it one iteration later
                        # keeps the PE from stalling.)
                        pvn, pex, prs = prev
                        ap_ps = mi_ps.tile([128, T], F32, tag="mi")
                        for t in range(T):
                            nc.tensor.matmul(
                                ap_ps[:, t:t + 1],
                                pex[0:1, t * 128:(t + 1) * 128],
                                ones[0:1, 0:1],
                                start=True, stop=True,
                            )
                        exT = sm_pool.tile([128, T], WD, tag="exT")
                        nc.vector.tensor_copy(exT[:], ap_ps[:])

                        cp = mi_ps.tile([1, D], F32, tag="mi")
                        for t in range(T):
                            nc.tensor.matmul(
                                cp[:], exT[:, t:t + 1], pvn[:, t, :],
                                start=(t == 0), stop=(t == T - 1),
                            )
                        ctx = sm_pool.tile([1, D], F32, tag="ctx")
                        nc.vector.tensor_scalar_mul(
                            ctx[:], cp[:], prs[0:1, 0:1])
                        nc.scalar.dma_start(out_d.ap()[b - 1:b, :], ctx[:])

                    if b == BC:
                        break

                    # --- softmax pieces (consumed next iteration) ---
                    neg_max = sm_pool.tile([1, 1], F32, tag="nm")
                    nc.vector.tensor_reduce(
                        neg_max[:], sc_row[:], axis=mybir.AxisListType.X,
                        op=mybir.AluOpType.max, negate=True)
                    ex_row = row_pool.tile([1, S], WD, tag="ex")
                    sumexp = sm_pool.tile([1, 1], F32, tag="se")
                    nc.scalar.activation(
                        ex_row[:], sc_row[:], AFT.Exp,
                        bias=neg_max[0:1, 0:1], accum_out=sumexp[0:1, 0:1])
                    rsum = sm_pool.tile([1, 1], F32, tag="rs")
                    nc.vector.reciprocal(rsum[:], sumexp[:])

                    prev = (vn, ex_row, rsum)

    nc.finalize()
    return nc


_NC_CACHE = None


def kernel(query, values, W1, b1, W2, b2, V, bv, **_):
    global _NC_CACHE
    query = np.asarray(query, dtype=np.float32)
    values = np.asarray(values, dtype=np.float32)
    W1 = np.asarray(W1, dtype=np.float32)
    W2 = np.asarray(W2, dtype=np.float32)
    b1 = np.ascontiguousarray(np.asarray(b1, dtype=np.float32))
    b2 = np.ascontiguousarray(np.asarray(b2, dtype=np.float32))
    V = np.asarray(V, dtype=np.float32).reshape(U)
    bv = np.ascontiguousarray(np.asarray(bv, dtype=np.float32).reshape(1, 1))

    # Layout/dtype prep (host): matmul-path operands in WD, both values
    # layouts contiguous.
    values_w = np.ascontiguousarray(values.astype(WD_NP))
    valuesT_w = np.ascontiguousarray(values_w.transpose(0, 2, 1))
    W1_w = np.ascontiguousarray(W1.astype(WD_NP))
    W2_w = np.ascontiguousarray(W2.astype(WD_NP))
    V_w = np.ascontiguousarray(V.astype(WD_NP))
    qT_w = np.ascontiguousarray(query.T.astype(WD_NP))

    if _NC_CACHE is None:
        _NC_CACHE = build_kernel()
    nc = _NC_CACHE

    in_maps = []
    for c in range(NCORES):
        sl = slice(c * BC, (c + 1) * BC)
        in_maps.append({
            "values": values_w[sl],
            "valuesT": valuesT_w[sl],
            "qT": np.ascontiguousarray(qT_w[:, sl]),
            "W1": W1_w, "W2": W2_w, "V": V_w,
            "b1": b1, "b2": b2, "bv": bv,
        })

    trace = os.environ.get("BAH_TRACE", "0") == "1"
    res = run_bass_kernel_spmd(
        nc, in_maps, core_ids=list(range(NCORES)), trace=trace)
    if trace:
        print(f"HW exec time: {res.exec_time_ns} ns "
              f"(mean {res.mean_exec_time_ns})")
    return np.concatenate([r["out"] for r in res.results], axis=0)


if __name__ == "__main__":
    rng = np.random.default_rng(0)
    inputs = {
        "query": rng.standard_normal((B, D), dtype=np.float32),
        "values": rng.standard_normal((B, S, D), dtype=np.float32),
        "W1": rng.standard_normal((D, U), dtype=np.float32) / np.sqrt(D),
        "b1": np.zeros(U, np.float32),
        "W2": rng.standard_normal((D, U), dtype=np.float32) / np.sqrt(D),
        "b2": np.zeros(U, np.float32),
        "V": rng.standard_normal((U, 1), dtype=np.float32) / np.sqrt(U),
        "bv": np.zeros(1, np.float32),
    }
    out = kernel(**inputs)
    print("out", out.shape, out.dtype, float(np.abs(out).max()))
